# revision 11
# baseline (speedup 1.0000x reference)
"""Causal multi-head attention kernel for TRN2 (8 NeuronCores, SPMD).

Problem: x[2,2048,1024], per-head W_qkv[16,1024,192], W_out[16,64,1024].
  qkv = einsum('bsd,ndh->bnsh', x, W_qkv); causal softmax attention per head;
  out.reshape(B,-1,S); einsum('bds,nhd->bsd', out, W_out).

Key observation: the final einsum does NOT contract d (it appears in both
operands and the output), so it reduces to
  result[b,s,d] = out_reshaped[b,d,s] * W_sum[d],  W_sum[d] = sum_{n,h} W_out[n,h,d]
i.e. a raw reshape + transpose + per-column scale. That part is pure data
movement and is done on the host; the device computes the attention.

Sharding: 2 batches x 16 heads = 32 jobs; core c handles batch c//4 and the
4 heads [4*(c%4), 4*(c%4)+4), as 2 head-pairs packed into 128 partitions.

Device per core (matmuls in fp16: full PE rate, ~16x better element
precision than bf16; PSUM accumulation is fp32):
  - QKV projection: psum = sum_d W2[d].T @ xT[d], 2 heads packed in M;
    two q-chunks share one weight load.
  - K^T kept head-packed [2H=128, S].  Q^T stored zero-padded per head
    (head a in rows 0:64 + zero rows, head b in rows 64:128 + zero rows)
    so each score matmul is a canonical full-K=128 matmul whose stationary
    operand (the packed K^T tile) is shared by both heads.
  - V^T -> [k, Va|1|Vb|1] tiles via PE transpose; the appended ones-column
    makes the AV matmul also produce the softmax denominator.
  - scores: S^T[k,q] tile pair for both heads in one 2-bank PSUM tile;
    ONE exp (ScalarE, scale=1/8, no max-subtraction needed: scores~N(0,1))
    per k-step; causal crossing tiles masked by a 0/1 fp16 multiply.
  - O'^T[65,q] += [V|1].T @ P^T accumulated over k: rows 0..63 attention
    output, row 64 denominator.  Causal column trimming on all of
    scores/exp/AV.
Host epilogue: normalize, reshape, scale by W_sum.
"""

import numpy as np

import concourse.bass as bass
import concourse.mybir as mybir
from concourse.tile import TileContext
from concourse.bass_utils import run_bass_kernel_spmd

F32 = mybir.dt.float32
MMD = mybir.dt.float16  # matmul operand dtype
NPD = np.float16

B, S, D, NH, HD = 2, 2048, 1024, 16, 64  # batch, seq, model, heads, head_dim
NCORES = 8
HPC = 4  # heads per core
NPAIR = 2  # head pairs per core
DT = D // 128  # 8 D-tiles
NQB = S // 512  # 4 q blocks
NKT = S // 128  # 16 k tiles
SCALE = 1.0 / np.sqrt(HD)


def _split_excess_waits(nc, limit=1):
    """This walrus build rejects >1 sync-wait per instruction; hoist extra
    waits onto preceding same-engine no-ops (identical blocking semantics)."""
    cnt = 0
    for fn in nc.m.functions:
        for blk in fn.blocks:
            out = []
            for inst in blk.instructions:
                si = inst.sync_info
                if si is not None and si.on_wait and len(si.on_wait) > limit:
                    waits = list(si.on_wait)
                    excess, keep = waits[:-limit], waits[-limit:]
                    for i in range(0, len(excess), limit):
                        nop = mybir.InstNoOp(
                            name=f"wsplit_{cnt}", ins=[], outs=[], engine=inst.engine
                        )
                        cnt += 1
                        nop.sync_info = mybir.SyncInfo(
                            on_wait=excess[i : i + limit], on_update=[]
                        )
                        out.append(nop)
                    inst.sync_info = mybir.SyncInfo(
                        on_wait=keep, on_update=list(si.on_update or [])
                    )
                out.append(inst)
            blk.instructions = out
    return cnt


def build_nc():
    nc = bass.Bass()
    xT = nc.declare_dram_parameter("xT", [D, S], MMD, isOutput=False)
    w = nc.declare_dram_parameter("w", [NPAIR, 3, DT, 128, 128], MMD, isOutput=False)
    mask = nc.declare_dram_parameter("mask", [4, 128, 1024], MMD, isOutput=False)
    ident = nc.declare_dram_parameter("ident", [128, 128], MMD, isOutput=False)
    out = nc.declare_dram_parameter("out", [65, HPC * S], F32, isOutput=True)

    with TileContext(nc) as tc:
        with (
            tc.tile_pool(name="persist", bufs=1) as pp,
            tc.tile_pool(name="psum", bufs=2, space="PSUM") as ps,
            tc.tile_pool(name="work", bufs=2) as pc,
        ):
            # ---- persistent SBUF tensors (Q^T and K^T head-packed [2H, S])
            qt2 = [
                pp.tile([128, S], MMD, tag=f"qt{p}", name=f"qtt{p}")
                for p in range(NPAIR)
            ]
            kt2 = [
                pp.tile([128, S], MMD, tag=f"kt{p}", name=f"ktt{p}")
                for p in range(NPAIR)
            ]
            v2e = [
                pp.tile([128, NKT, 130], MMD, tag=f"v2e{p}", name=f"v2e{p}")
                for p in range(NPAIR)
            ]
            mask_sb = pp.tile([128, 4, 1024], MMD, tag="mask", name="mask_sb")
            ident_sb = pp.tile([128, 128], MMD, tag="ident", name="ident_sb")
            xt_sb = pp.tile([128, DT, S], MMD, tag="xt", name="xt_sb")
            w_sb = pp.tile([128, NPAIR * 3 * DT, 128], MMD, tag="w", name="w_sb")
            vt = [
                pp.tile([128, S], MMD, tag=f"vt{p}", name=f"vt{p}")
                for p in range(NPAIR)
            ]

            # DMA order = consumption order: pair-0 weights + first xT half
            # gate the first projection matmuls.
            w_v = w.rearrange("a t d k m -> k (a t d) m")
            xt_v = xT.rearrange("(dt p) s -> p dt s", p=128)
            nc.sync.dma_start(out=w_sb[:, 0 : 3 * DT, :], in_=w_v[:, 0 : 3 * DT, :])
            for c4 in range(2):
                cs = slice(c4 * 512, (c4 + 1) * 512)
                nc.sync.dma_start(out=xt_sb[:, :, cs], in_=xt_v[:, :, cs])
            nc.sync.dma_start(out=ident_sb[:], in_=ident[:])
            nc.sync.dma_start(
                out=w_sb[:, 3 * DT : 6 * DT, :], in_=w_v[:, 3 * DT : 6 * DT, :]
            )
            for c4 in range(2, 4):
                cs = slice(c4 * 512, (c4 + 1) * 512)
                nc.sync.dma_start(out=xt_sb[:, :, cs], in_=xt_v[:, :, cs])
            nc.sync.dma_start(out=mask_sb[:], in_=mask.rearrange("r k q -> k r q"))
            for p in range(NPAIR):
                nc.vector.memset(v2e[p][:, :, 64], 1.0)
                nc.vector.memset(v2e[p][:, :, 129], 1.0)

            def proj_chunk(qcp):
                """Project q-columns [qcp*1024, (qcp+1)*1024) for all pairs;
                transpose the V k-tiles of that chunk."""
                for p in range(NPAIR):
                    for t in range(3):
                        acc2 = ps.tile([128, 1024], F32, tag="mm", name="acc2")
                        for d in range(DT):
                            wsl = w_sb[:, (p * 3 + t) * DT + d, :]
                            for h in range(2):
                                qc = 2 * qcp + h
                                nc.tensor.matmul(
                                    acc2[:, h * 512 : (h + 1) * 512],
                                    wsl,
                                    xt_sb[:, d, qc * 512 : (qc + 1) * 512],
                                    start=(d == 0),
                                    stop=(d == DT - 1),
                                )
                        csl = slice(qcp * 1024, (qcp + 1) * 1024)
                        if t == 0:
                            nc.vector.tensor_copy(qt2[p][:, csl], acc2[:])
                        elif t == 1:
                            nc.vector.tensor_copy(kt2[p][:, csl], acc2[:])
                        else:  # V -> transpose this chunk's k-tiles now
                            nc.vector.tensor_copy(vt[p][:, csl], acc2[:])
                            for k in range(8 * qcp, 8 * qcp + 8):
                                tp = ps.tile([128, 1024], MMD, tag="mm", name="tp")
                                nc.tensor.transpose(
                                    tp[:, 0:128],
                                    vt[p][:, k * 128 : (k + 1) * 128],
                                    ident_sb[:],
                                )
                                nc.vector.tensor_copy(
                                    v2e[p][:, k, 0:64], tp[:, 0:64]
                                )
                                nc.vector.tensor_copy(
                                    v2e[p][:, k, 65:129], tp[:, 64:128]
                                )

            warm_ps = ps.tile([1, 16], F32, tag="warm", name="warm_ps", bufs=1)

            def ham_warm():
                # tiny full-K matmul: transpose-mode and row-tiled matmuls do
                # not register as PE activity for the HAM clock gate; this
                # keeps the PE at 2.4GHz through the attention phase
                nc.tensor.matmul(
                    warm_ps[0:1, 0:1],
                    ident_sb[:, 0:1],
                    ident_sb[:, 0:1],
                    start=True,
                    stop=True,
                    skip_group_check=True,
                )

            def attention(p, qb):
                nk = 4 * (qb + 1)  # causal: k tiles 0..nk-1
                oa = ps.tile([65, 512], F32, tag="acca", name="oa", bufs=1)
                ob = ps.tile([65, 512], F32, tag="accb", name="ob", bufs=1)
                # software pipeline: scores(k) ... AV(k-1) so the PE never
                # stalls on the ScalarE exp of the current tile
                pt_tiles = [None] * nk

                def scores(k):
                    q0 = max(0, 128 * (k - 4 * qb))
                    s2 = ps.tile([128, 1024], F32, tag="mm", name="s2")
                    qsl = slice(qb * 512 + q0, (qb + 1) * 512)
                    for e in range(2):
                        rows = slice(64 * e, 64 * e + 64)
                        nc.tensor.matmul(
                            s2[:, e * 512 + q0 : (e + 1) * 512],
                            kt2[p][rows, k * 128 : (k + 1) * 128],
                            qt2[p][rows, qsl],
                            start=True,
                            stop=True,
                            tile_position=(64 * e, 0),
                        )
                    pt2 = pc.tile([128, 1024], MMD, tag="pt", name="pt2", bufs=4)
                    nc.scalar.activation(
                        pt2[:, q0:1024],
                        s2[:, q0:1024],
                        mybir.ActivationFunctionType.Exp,
                        scale=float(SCALE),
                    )
                    rel = k - 4 * qb
                    if rel >= 0:  # diagonal-crossing: 0/1 mask
                        nc.vector.tensor_mul(
                            pt2[:, q0:1024],
                            pt2[:, q0:1024],
                            mask_sb[:, rel, q0:1024],
                        )
                    pt_tiles[k] = (pt2, q0)

                def av(k):
                    pt2, q0 = pt_tiles[k]
                    nc.tensor.matmul(
                        oa[:, q0:512],
                        v2e[p][:, k, 0:65],
                        pt2[:, q0:512],
                        start=(k == 0),
                        stop=(k == nk - 1),
                    )
                    nc.tensor.matmul(
                        ob[:, q0:512],
                        v2e[p][:, k, 65:130],
                        pt2[:, 512 + q0 : 1024],
                        start=(k == 0),
                        stop=(k == nk - 1),
                    )
                    pt_tiles[k] = None

                for k in range(nk):
                    scores(k)
                    if k % 2 == 0:
                        ham_warm()
                    if k > 0:
                        av(k - 1)
                av(nk - 1)

                stage = pc.tile([65, 2, 512], F32, tag="stage", name="stage")
                nc.vector.tensor_copy(stage[:, 0, :], oa[:])
                nc.vector.tensor_copy(stage[:, 1, :], ob[:])
                nc.sync.dma_start(
                    out=out.rearrange("h (nl q) -> h nl q", nl=HPC)[
                        :, 2 * p : 2 * p + 2, qb * 512 : (qb + 1) * 512
                    ],
                    in_=stage[:],
                )

            # interleave: attention on early q-blocks overlaps the second
            # projection chunk (its exp work runs under proj matmuls)
            proj_chunk(0)
            attention(0, 1)
            attention(1, 1)
            proj_chunk(1)
            attention(0, 3)
            attention(1, 3)
            attention(0, 2)
            attention(1, 2)
            attention(0, 0)
            attention(1, 0)

    _split_excess_waits(nc)
    return nc


_NC_CACHE = None


def _get_nc():
    global _NC_CACHE
    if _NC_CACHE is None:
        _NC_CACHE = build_nc()
    return _NC_CACHE


def _host_inputs(x, W_qkv):
    """Per-core input maps."""
    xT = [np.ascontiguousarray(x[b].T).astype(NPD) for b in range(B)]  # [D, S]
    # w[pair, t, dtile, 128, 128]: cols 0:64 head a, 64:128 head b
    Wr = np.ascontiguousarray(W_qkv.reshape(NH, DT, 128, 3, HD))
    ki = np.arange(128)[:, None]
    qj = np.arange(512)[None, :]
    m1 = np.zeros((4, 128, 512), dtype=np.float32)
    for r in range(4):
        m1[r] = (ki <= qj - 128 * r).astype(np.float32)
    mask = np.concatenate([m1, m1], axis=2).astype(NPD)  # [4, 128, 1024]
    ident = np.eye(128, dtype=np.float32).astype(NPD)
    in_maps = []
    for c in range(NCORES):
        b = c // 4
        h0 = 4 * (c % 4)
        w = np.empty((NPAIR, 3, DT, 128, 128), dtype=np.float32)
        for p in range(NPAIR):
            ha, hb = h0 + 2 * p, h0 + 2 * p + 1
            for t in range(3):
                w[p, t, :, :, 0:64] = Wr[ha, :, :, t, :]
                w[p, t, :, :, 64:128] = Wr[hb, :, :, t, :]
        in_maps.append(
            {"xT": xT[b], "w": w.astype(NPD), "mask": mask, "ident": ident}
        )
    return in_maps


def _host_epilogue(results, W_out):
    W_sum = W_out.sum(axis=(0, 1)).astype(np.float32)  # [D]
    O = np.empty((B, NH, S, HD), dtype=np.float32)
    for c in range(NCORES):
        o = results[c]["out"]  # [65, 4*2048]
        b = c // 4
        h0 = 4 * (c % 4)
        body = o[0:64].reshape(64, HPC, S)  # [h, nl, s]
        den = o[64].reshape(HPC, S)  # [nl, s]
        O[b, h0 : h0 + HPC] = body.transpose(1, 2, 0) / den[:, :, None]
    out2 = O.reshape(B, D, S)  # raw row-major reshape, as in the reference
    return np.ascontiguousarray(
        out2.transpose(0, 2, 1) * W_sum[None, None, :]
    ).astype(np.float32)


def _run(x, W_qkv, W_out, trace=False):
    nc = _get_nc()
    in_maps = _host_inputs(x, W_qkv)
    res = run_bass_kernel_spmd(
        nc,
        in_maps,
        list(range(NCORES)),
        trace=trace,
        trace_cores=list(range(NCORES)) if trace else None,
    )
    return _host_epilogue(res.results, W_out), res


def kernel(x, W_qkv, W_out):
    x = np.asarray(x, dtype=np.float32)
    W_qkv = np.asarray(W_qkv, dtype=np.float32)
    W_out = np.asarray(W_out, dtype=np.float32)
    out, _ = _run(x, W_qkv, W_out, trace=False)
    return out


def kernel_traced(x, W_qkv, W_out):
    out, res = _run(
        np.asarray(x, np.float32),
        np.asarray(W_qkv, np.float32),
        np.asarray(W_out, np.float32),
        trace=True,
    )
    return out, res


# revision 12
# speedup vs baseline: 1.0758x; 1.0758x over previous
"""Causal multi-head attention kernel for TRN2 (8 NeuronCores, SPMD).

Problem: x[2,2048,1024], per-head W_qkv[16,1024,192], W_out[16,64,1024].
  qkv = einsum('bsd,ndh->bnsh', x, W_qkv); causal softmax attention per head;
  out.reshape(B,-1,S); einsum('bds,nhd->bsd', out, W_out).

Key observation: the final einsum does NOT contract d (it appears in both
operands and the output), so it reduces to
  result[b,s,d] = out_reshaped[b,d,s] * W_sum[d],  W_sum[d] = sum_{n,h} W_out[n,h,d]
i.e. a raw reshape + transpose + per-column scale. That part is pure data
movement and is done on the host; the device computes the attention.

Sharding: 2 batches x 16 heads = 32 jobs; core c handles batch c//4 and the
4 heads [4*(c%4), 4*(c%4)+4), as 2 head-pairs packed into 128 partitions.

Device per core (matmuls in fp16: full PE rate, ~16x better element
precision than bf16; PSUM accumulation is fp32):
  - QKV projection: psum = sum_d W2[d].T @ xT[d], 2 heads packed in M;
    two q-chunks share one weight load.
  - K^T kept head-packed [2H=128, S].  Q^T stored zero-padded per head
    (head a in rows 0:64 + zero rows, head b in rows 64:128 + zero rows)
    so each score matmul is a canonical full-K=128 matmul whose stationary
    operand (the packed K^T tile) is shared by both heads.
  - V^T -> [k, Va|1|Vb|1] tiles via PE transpose; the appended ones-column
    makes the AV matmul also produce the softmax denominator.
  - scores: S^T[k,q] tile pair for both heads in one 2-bank PSUM tile;
    ONE exp (ScalarE, scale=1/8, no max-subtraction needed: scores~N(0,1))
    per k-step; causal crossing tiles masked by a 0/1 fp16 multiply.
  - O'^T[65,q] += [V|1].T @ P^T accumulated over k: rows 0..63 attention
    output, row 64 denominator.  Causal column trimming on all of
    scores/exp/AV.
Host epilogue: normalize, reshape, scale by W_sum.
"""

import numpy as np

import concourse.bass as bass
import concourse.mybir as mybir
from concourse.tile import TileContext
from concourse.bass_utils import run_bass_kernel_spmd

F32 = mybir.dt.float32
MMD = mybir.dt.float16  # matmul operand dtype
NPD = np.float16

B, S, D, NH, HD = 2, 2048, 1024, 16, 64  # batch, seq, model, heads, head_dim
NCORES = 8
HPC = 4  # heads per core
NPAIR = 2  # head pairs per core
DT = D // 128  # 8 D-tiles
NQB = S // 512  # 4 q blocks
NKT = S // 128  # 16 k tiles
SCALE = 1.0 / np.sqrt(HD)


def _split_excess_waits(nc, limit=1):
    """This walrus build rejects >1 sync-wait per instruction; hoist extra
    waits onto preceding same-engine no-ops (identical blocking semantics)."""
    cnt = 0
    for fn in nc.m.functions:
        for blk in fn.blocks:
            out = []
            for inst in blk.instructions:
                si = inst.sync_info
                if si is not None and si.on_wait and len(si.on_wait) > limit:
                    waits = list(si.on_wait)
                    excess, keep = waits[:-limit], waits[-limit:]
                    for i in range(0, len(excess), limit):
                        nop = mybir.InstNoOp(
                            name=f"wsplit_{cnt}", ins=[], outs=[], engine=inst.engine
                        )
                        cnt += 1
                        nop.sync_info = mybir.SyncInfo(
                            on_wait=excess[i : i + limit], on_update=[]
                        )
                        out.append(nop)
                    inst.sync_info = mybir.SyncInfo(
                        on_wait=keep, on_update=list(si.on_update or [])
                    )
                out.append(inst)
            blk.instructions = out
    return cnt


def build_nc():
    nc = bass.Bass()
    xT = nc.declare_dram_parameter("xT", [D, S], MMD, isOutput=False)
    w = nc.declare_dram_parameter("w", [NPAIR, 3, DT, 128, 128], MMD, isOutput=False)
    mask = nc.declare_dram_parameter("mask", [4, 128, 1024], MMD, isOutput=False)
    ident = nc.declare_dram_parameter("ident", [128, 128], MMD, isOutput=False)
    out = nc.declare_dram_parameter("out", [65, HPC * S], F32, isOutput=True)

    with TileContext(nc) as tc:
        with (
            tc.tile_pool(name="persist", bufs=1) as pp,
            tc.tile_pool(name="psum", bufs=3, space="PSUM") as ps,
            tc.tile_pool(name="work", bufs=2) as pc,
        ):
            # ---- persistent SBUF tensors (Q^T and K^T head-packed [2H, S])
            qt2 = [
                pp.tile([128, S], MMD, tag=f"qt{p}", name=f"qtt{p}")
                for p in range(NPAIR)
            ]
            kt2 = [
                pp.tile([128, S], MMD, tag=f"kt{p}", name=f"ktt{p}")
                for p in range(NPAIR)
            ]
            v2e = [
                pp.tile([128, NKT, 130], MMD, tag=f"v2e{p}", name=f"v2e{p}")
                for p in range(NPAIR)
            ]
            mask_sb = pp.tile([128, 4, 1024], MMD, tag="mask", name="mask_sb")
            ident_sb = pp.tile([128, 128], MMD, tag="ident", name="ident_sb")
            xt_sb = pp.tile([128, DT, S], MMD, tag="xt", name="xt_sb")
            w_sb = pp.tile([128, NPAIR * 3 * DT, 128], MMD, tag="w", name="w_sb")
            vt = [
                pp.tile([128, S], MMD, tag=f"vt{p}", name=f"vt{p}")
                for p in range(NPAIR)
            ]

            # DMA order = consumption order: pair-0 weights + first xT half
            # gate the first projection matmuls.
            w_v = w.rearrange("a t d k m -> k (a t d) m")
            xt_v = xT.rearrange("(dt p) s -> p dt s", p=128)
            nc.sync.dma_start(out=w_sb[:, 0 : 3 * DT, :], in_=w_v[:, 0 : 3 * DT, :])
            for c4 in range(2):
                cs = slice(c4 * 512, (c4 + 1) * 512)
                nc.sync.dma_start(out=xt_sb[:, :, cs], in_=xt_v[:, :, cs])
            nc.sync.dma_start(out=ident_sb[:], in_=ident[:])
            nc.sync.dma_start(
                out=w_sb[:, 3 * DT : 6 * DT, :], in_=w_v[:, 3 * DT : 6 * DT, :]
            )
            for c4 in range(2, 4):
                cs = slice(c4 * 512, (c4 + 1) * 512)
                nc.sync.dma_start(out=xt_sb[:, :, cs], in_=xt_v[:, :, cs])
            nc.sync.dma_start(out=mask_sb[:], in_=mask.rearrange("r k q -> k r q"))
            for p in range(NPAIR):
                nc.vector.memset(v2e[p][:, :, 64], 1.0)
                nc.vector.memset(v2e[p][:, :, 129], 1.0)

            def proj_chunk(qcp):
                """Project q-columns [qcp*1024, (qcp+1)*1024) for all pairs;
                transpose the V k-tiles of that chunk."""
                for p in range(NPAIR):
                    for t in range(3):
                        acc2 = ps.tile([128, 1024], F32, tag="mm", name="acc2")
                        for d in range(DT):
                            wsl = w_sb[:, (p * 3 + t) * DT + d, :]
                            for h in range(2):
                                qc = 2 * qcp + h
                                nc.tensor.matmul(
                                    acc2[:, h * 512 : (h + 1) * 512],
                                    wsl,
                                    xt_sb[:, d, qc * 512 : (qc + 1) * 512],
                                    start=(d == 0),
                                    stop=(d == DT - 1),
                                )
                        csl = slice(qcp * 1024, (qcp + 1) * 1024)
                        if t == 0:
                            nc.vector.tensor_copy(qt2[p][:, csl], acc2[:])
                        elif t == 1:
                            nc.vector.tensor_copy(kt2[p][:, csl], acc2[:])
                        else:  # V -> transpose this chunk's k-tiles now
                            nc.vector.tensor_copy(vt[p][:, csl], acc2[:])
                            for k in range(8 * qcp, 8 * qcp + 8):
                                tp = ps.tile([128, 1024], MMD, tag="mm", name="tp")
                                nc.tensor.transpose(
                                    tp[:, 0:128],
                                    vt[p][:, k * 128 : (k + 1) * 128],
                                    ident_sb[:],
                                )
                                nc.vector.tensor_copy(
                                    v2e[p][:, k, 0:64], tp[:, 0:64]
                                )
                                nc.vector.tensor_copy(
                                    v2e[p][:, k, 65:129], tp[:, 64:128]
                                )

            def ham_warm():
                # tiny full-K matmul: transpose-mode and row-tiled matmuls do
                # not register as PE activity for the HAM clock gate
                wps = ps.tile([1, 16], F32, tag="mm", name="wps")
                nc.tensor.matmul(
                    wps[0:1, 0:1],
                    ident_sb[:, 0:1],
                    ident_sb[:, 0:1],
                    start=True,
                    stop=True,
                    skip_group_check=True,
                )

            def attention(p, qb):
                nk = 4 * (qb + 1)  # causal: k tiles 0..nk-1
                oa = ps.tile([65, 512], F32, tag="acca", name="oa", bufs=1)
                ob = ps.tile([65, 512], F32, tag="accb", name="ob", bufs=1)
                # software pipeline: scores(k) ... AV(k-1) so the PE never
                # stalls on the ScalarE exp of the current tile
                pt_tiles = [None] * nk

                def scores(k):
                    q0 = max(0, 128 * (k - 4 * qb))
                    s2 = ps.tile([128, 1024], F32, tag="mm", name="s2")
                    qsl = slice(qb * 512 + q0, (qb + 1) * 512)
                    for e in range(2):
                        rows = slice(64 * e, 64 * e + 64)
                        nc.tensor.matmul(
                            s2[:, e * 512 + q0 : (e + 1) * 512],
                            kt2[p][rows, k * 128 : (k + 1) * 128],
                            qt2[p][rows, qsl],
                            start=True,
                            stop=True,
                            tile_position=(64 * e, 0),
                        )
                    pt2 = pc.tile([128, 1024], MMD, tag="pt", name="pt2", bufs=4)
                    nc.scalar.activation(
                        pt2[:, q0:1024],
                        s2[:, q0:1024],
                        mybir.ActivationFunctionType.Exp,
                        scale=float(SCALE),
                    )
                    rel = k - 4 * qb
                    if rel >= 0:  # diagonal-crossing: 0/1 mask
                        nc.vector.tensor_mul(
                            pt2[:, q0:1024],
                            pt2[:, q0:1024],
                            mask_sb[:, rel, q0:1024],
                        )
                    pt_tiles[k] = (pt2, q0)

                def av(k):
                    pt2, q0 = pt_tiles[k]
                    nc.tensor.matmul(
                        oa[:, q0:512],
                        v2e[p][:, k, 0:65],
                        pt2[:, q0:512],
                        start=(k == 0),
                        stop=(k == nk - 1),
                    )
                    nc.tensor.matmul(
                        ob[:, q0:512],
                        v2e[p][:, k, 65:130],
                        pt2[:, 512 + q0 : 1024],
                        start=(k == 0),
                        stop=(k == nk - 1),
                    )
                    pt_tiles[k] = None

                for k in range(nk):
                    scores(k)
                    if k % 2 == 0:
                        ham_warm()
                    if k > 0:
                        av(k - 1)
                av(nk - 1)

                stage = pc.tile([65, 2, 512], F32, tag="stage", name="stage")
                nc.vector.tensor_copy(stage[:, 0, :], oa[:])
                nc.vector.tensor_copy(stage[:, 1, :], ob[:])
                nc.sync.dma_start(
                    out=out.rearrange("h (nl q) -> h nl q", nl=HPC)[
                        :, 2 * p : 2 * p + 2, qb * 512 : (qb + 1) * 512
                    ],
                    in_=stage[:],
                )

            # interleave: attention on early q-blocks overlaps the second
            # projection chunk (its exp work runs under proj matmuls)
            proj_chunk(0)
            attention(0, 1)
            attention(1, 1)
            attention(0, 0)
            attention(1, 0)
            proj_chunk(1)
            attention(0, 3)
            attention(1, 3)
            attention(0, 2)
            attention(1, 2)

    _split_excess_waits(nc)
    return nc


_NC_CACHE = None


def _get_nc():
    global _NC_CACHE
    if _NC_CACHE is None:
        _NC_CACHE = build_nc()
    return _NC_CACHE


def _host_inputs(x, W_qkv):
    """Per-core input maps."""
    xT = [np.ascontiguousarray(x[b].T).astype(NPD) for b in range(B)]  # [D, S]
    # w[pair, t, dtile, 128, 128]: cols 0:64 head a, 64:128 head b
    Wr = np.ascontiguousarray(W_qkv.reshape(NH, DT, 128, 3, HD))
    ki = np.arange(128)[:, None]
    qj = np.arange(512)[None, :]
    m1 = np.zeros((4, 128, 512), dtype=np.float32)
    for r in range(4):
        m1[r] = (ki <= qj - 128 * r).astype(np.float32)
    mask = np.concatenate([m1, m1], axis=2).astype(NPD)  # [4, 128, 1024]
    ident = np.eye(128, dtype=np.float32).astype(NPD)
    in_maps = []
    for c in range(NCORES):
        b = c // 4
        h0 = 4 * (c % 4)
        w = np.empty((NPAIR, 3, DT, 128, 128), dtype=np.float32)
        for p in range(NPAIR):
            ha, hb = h0 + 2 * p, h0 + 2 * p + 1
            for t in range(3):
                w[p, t, :, :, 0:64] = Wr[ha, :, :, t, :]
                w[p, t, :, :, 64:128] = Wr[hb, :, :, t, :]
        in_maps.append(
            {"xT": xT[b], "w": w.astype(NPD), "mask": mask, "ident": ident}
        )
    return in_maps


def _host_epilogue(results, W_out):
    W_sum = W_out.sum(axis=(0, 1)).astype(np.float32)  # [D]
    O = np.empty((B, NH, S, HD), dtype=np.float32)
    for c in range(NCORES):
        o = results[c]["out"]  # [65, 4*2048]
        b = c // 4
        h0 = 4 * (c % 4)
        body = o[0:64].reshape(64, HPC, S)  # [h, nl, s]
        den = o[64].reshape(HPC, S)  # [nl, s]
        O[b, h0 : h0 + HPC] = body.transpose(1, 2, 0) / den[:, :, None]
    out2 = O.reshape(B, D, S)  # raw row-major reshape, as in the reference
    return np.ascontiguousarray(
        out2.transpose(0, 2, 1) * W_sum[None, None, :]
    ).astype(np.float32)


def _run(x, W_qkv, W_out, trace=False):
    nc = _get_nc()
    in_maps = _host_inputs(x, W_qkv)
    res = run_bass_kernel_spmd(
        nc,
        in_maps,
        list(range(NCORES)),
        trace=trace,
        trace_cores=list(range(NCORES)) if trace else None,
    )
    return _host_epilogue(res.results, W_out), res


def kernel(x, W_qkv, W_out):
    x = np.asarray(x, dtype=np.float32)
    W_qkv = np.asarray(W_qkv, dtype=np.float32)
    W_out = np.asarray(W_out, dtype=np.float32)
    out, _ = _run(x, W_qkv, W_out, trace=False)
    return out


def kernel_traced(x, W_qkv, W_out):
    out, res = _run(
        np.asarray(x, np.float32),
        np.asarray(W_qkv, np.float32),
        np.asarray(W_out, np.float32),
        trace=True,
    )
    return out, res


# revision 14
# speedup vs baseline: 1.1290x; 1.0494x over previous
"""Causal multi-head attention kernel for TRN2 (8 NeuronCores, SPMD).

Problem: x[2,2048,1024], per-head W_qkv[16,1024,192], W_out[16,64,1024].
  qkv = einsum('bsd,ndh->bnsh', x, W_qkv); causal softmax attention per head;
  out.reshape(B,-1,S); einsum('bds,nhd->bsd', out, W_out).

Key observation: the final einsum does NOT contract d (it appears in both
operands and the output), so it reduces to
  result[b,s,d] = out_reshaped[b,d,s] * W_sum[d],  W_sum[d] = sum_{n,h} W_out[n,h,d]
i.e. a raw reshape + transpose + per-column scale. That part is pure data
movement and is done on the host; the device computes the attention.

Sharding: 2 batches x 16 heads = 32 jobs; core c handles batch c//4 and the
4 heads [4*(c%4), 4*(c%4)+4), as 2 head-pairs packed into 128 partitions.

Device per core (matmuls in fp16: full PE rate, ~16x better element
precision than bf16; PSUM accumulation is fp32):
  - QKV projection: psum = sum_d W2[d].T @ xT[d], 2 heads packed in M;
    two q-chunks share one weight load.
  - K^T kept head-packed [2H=128, S].  Q^T stored zero-padded per head
    (head a in rows 0:64 + zero rows, head b in rows 64:128 + zero rows)
    so each score matmul is a canonical full-K=128 matmul whose stationary
    operand (the packed K^T tile) is shared by both heads.
  - V^T -> [k, Va|1|Vb|1] tiles via PE transpose; the appended ones-column
    makes the AV matmul also produce the softmax denominator.
  - scores: S^T[k,q] tile pair for both heads in one 2-bank PSUM tile;
    ONE exp (ScalarE, scale=1/8, no max-subtraction needed: scores~N(0,1))
    per k-step; causal crossing tiles masked by a 0/1 fp16 multiply.
  - O'^T[65,q] += [V|1].T @ P^T accumulated over k: rows 0..63 attention
    output, row 64 denominator.  Causal column trimming on all of
    scores/exp/AV.
Host epilogue: normalize, reshape, scale by W_sum.
"""

import numpy as np

import concourse.bass as bass
import concourse.mybir as mybir
from concourse.tile import TileContext
from concourse.bass_utils import run_bass_kernel_spmd

F32 = mybir.dt.float32
MMD = mybir.dt.float16  # matmul operand dtype
NPD = np.float16

B, S, D, NH, HD = 2, 2048, 1024, 16, 64  # batch, seq, model, heads, head_dim
NCORES = 8
HPC = 4  # heads per core
NPAIR = 2  # head pairs per core
DT = D // 128  # 8 D-tiles
NQB = S // 512  # 4 q blocks
NKT = S // 128  # 16 k tiles
SCALE = 1.0 / np.sqrt(HD)


def _split_excess_waits(nc, limit=1):
    """This walrus build rejects >1 sync-wait per instruction; hoist extra
    waits onto preceding same-engine no-ops (identical blocking semantics)."""
    cnt = 0
    for fn in nc.m.functions:
        for blk in fn.blocks:
            out = []
            for inst in blk.instructions:
                si = inst.sync_info
                if si is not None and si.on_wait and len(si.on_wait) > limit:
                    waits = list(si.on_wait)
                    excess, keep = waits[:-limit], waits[-limit:]
                    for i in range(0, len(excess), limit):
                        nop = mybir.InstNoOp(
                            name=f"wsplit_{cnt}", ins=[], outs=[], engine=inst.engine
                        )
                        cnt += 1
                        nop.sync_info = mybir.SyncInfo(
                            on_wait=excess[i : i + limit], on_update=[]
                        )
                        out.append(nop)
                    inst.sync_info = mybir.SyncInfo(
                        on_wait=keep, on_update=list(si.on_update or [])
                    )
                out.append(inst)
            blk.instructions = out
    return cnt


def build_nc():
    nc = bass.Bass()
    xT = nc.declare_dram_parameter("xT", [D, S], MMD, isOutput=False)
    w = nc.declare_dram_parameter("w", [NPAIR, 3, DT, 128, 128], MMD, isOutput=False)
    mask = nc.declare_dram_parameter("mask", [4, 128, 1024], MMD, isOutput=False)
    ident = nc.declare_dram_parameter("ident", [128, 128], MMD, isOutput=False)
    out = nc.declare_dram_parameter("out", [65, HPC * S], F32, isOutput=True)

    with TileContext(nc) as tc:
        with (
            tc.tile_pool(name="persist", bufs=1) as pp,
            tc.tile_pool(name="psum", bufs=3, space="PSUM") as ps,
            tc.tile_pool(name="work", bufs=2) as pc,
        ):
            # ---- persistent SBUF tensors (Q^T and K^T head-packed [2H, S])
            qt2 = [
                pp.tile([128, S], MMD, tag=f"qt{p}", name=f"qtt{p}")
                for p in range(NPAIR)
            ]
            kt2 = [
                pp.tile([128, S], MMD, tag=f"kt{p}", name=f"ktt{p}")
                for p in range(NPAIR)
            ]
            v2e = [
                pp.tile([128, NKT, 130], MMD, tag=f"v2e{p}", name=f"v2e{p}")
                for p in range(NPAIR)
            ]
            mask_sb = pp.tile([128, 4, 1024], MMD, tag="mask", name="mask_sb")
            ident_sb = pp.tile([128, 128], MMD, tag="ident", name="ident_sb")
            xt_sb = pp.tile([128, DT, S], MMD, tag="xt", name="xt_sb")
            w_sb = pp.tile([128, NPAIR * 3 * DT, 128], MMD, tag="w", name="w_sb")
            vt = [
                pp.tile([128, S], MMD, tag=f"vt{p}", name=f"vt{p}")
                for p in range(NPAIR)
            ]

            # DMA order = consumption order: pair-0 weights + first xT half
            # gate the first projection matmuls.
            w_v = w.rearrange("a t d k m -> k (a t d) m")
            xt_v = xT.rearrange("(dt p) s -> p dt s", p=128)
            nc.sync.dma_start(out=w_sb[:, 0 : 3 * DT, :], in_=w_v[:, 0 : 3 * DT, :])
            for c4 in range(2):
                cs = slice(c4 * 512, (c4 + 1) * 512)
                nc.sync.dma_start(out=xt_sb[:, :, cs], in_=xt_v[:, :, cs])
            nc.sync.dma_start(out=ident_sb[:], in_=ident[:])
            nc.sync.dma_start(
                out=w_sb[:, 3 * DT : 6 * DT, :], in_=w_v[:, 3 * DT : 6 * DT, :]
            )
            for c4 in range(2, 4):
                cs = slice(c4 * 512, (c4 + 1) * 512)
                nc.sync.dma_start(out=xt_sb[:, :, cs], in_=xt_v[:, :, cs])
            nc.sync.dma_start(out=mask_sb[:], in_=mask.rearrange("r k q -> k r q"))
            for p in range(NPAIR):
                nc.vector.memset(v2e[p][:, :, 64], 1.0)
                nc.vector.memset(v2e[p][:, :, 129], 1.0)

            def proj_chunk(qcp):
                """Project q-columns [qcp*1024, (qcp+1)*1024) for all pairs;
                transpose the V k-tiles of that chunk."""
                for p in range(NPAIR):
                    for t in range(3):
                        acc2 = ps.tile([128, 1024], F32, tag="mm", name="acc2")
                        for d in range(DT):
                            wsl = w_sb[:, (p * 3 + t) * DT + d, :]
                            for h in range(2):
                                qc = 2 * qcp + h
                                nc.tensor.matmul(
                                    acc2[:, h * 512 : (h + 1) * 512],
                                    wsl,
                                    xt_sb[:, d, qc * 512 : (qc + 1) * 512],
                                    start=(d == 0),
                                    stop=(d == DT - 1),
                                )
                        csl = slice(qcp * 1024, (qcp + 1) * 1024)
                        if t == 0:
                            nc.vector.tensor_copy(qt2[p][:, csl], acc2[:])
                        elif t == 1:
                            nc.vector.tensor_copy(kt2[p][:, csl], acc2[:])
                        else:  # V -> transpose this chunk's k-tiles now
                            nc.vector.tensor_copy(vt[p][:, csl], acc2[:])
                            for k in range(8 * qcp, 8 * qcp + 8):
                                tp = ps.tile([128, 512], MMD, tag="acca", name="tp", bufs=1)
                                nc.tensor.transpose(
                                    tp[:, 0:128],
                                    vt[p][:, k * 128 : (k + 1) * 128],
                                    ident_sb[:],
                                )
                                nc.vector.tensor_copy(
                                    v2e[p][:, k, 0:64], tp[:, 0:64]
                                )
                                nc.vector.tensor_copy(
                                    v2e[p][:, k, 65:129], tp[:, 64:128]
                                )

            def attention(p, qb):
                nk = 4 * (qb + 1)  # causal: k tiles 0..nk-1
                oa = ps.tile([65, 512], F32, tag="acca", name="oa", bufs=1)
                ob = ps.tile([65, 512], F32, tag="accb", name="ob", bufs=1)
                # software pipeline: scores(k) ... AV(k-1) so the PE never
                # stalls on the ScalarE exp of the current tile
                pt_tiles = [None] * nk

                def scores(k):
                    q0 = max(0, 128 * (k - 4 * qb))
                    s2 = ps.tile([128, 1024], F32, tag="mm", name="s2")
                    qsl = slice(qb * 512 + q0, (qb + 1) * 512)
                    for e in range(2):
                        rows = slice(64 * e, 64 * e + 64)
                        nc.tensor.matmul(
                            s2[:, e * 512 + q0 : (e + 1) * 512],
                            kt2[p][rows, k * 128 : (k + 1) * 128],
                            qt2[p][rows, qsl],
                            start=True,
                            stop=True,
                            tile_position=(64 * e, 0),
                        )
                    pt2 = pc.tile([128, 1024], MMD, tag="pt", name="pt2", bufs=4)
                    nc.scalar.activation(
                        pt2[:, q0:1024],
                        s2[:, q0:1024],
                        mybir.ActivationFunctionType.Exp,
                        scale=float(SCALE),
                    )
                    rel = k - 4 * qb
                    if rel >= 0:  # diagonal-crossing: 0/1 mask
                        nc.vector.tensor_mul(
                            pt2[:, q0:1024],
                            pt2[:, q0:1024],
                            mask_sb[:, rel, q0:1024],
                        )
                    pt_tiles[k] = (pt2, q0)

                def av(k):
                    pt2, q0 = pt_tiles[k]
                    nc.tensor.matmul(
                        oa[:, q0:512],
                        v2e[p][:, k, 0:65],
                        pt2[:, q0:512],
                        start=(k == 0),
                        stop=(k == nk - 1),
                    )
                    nc.tensor.matmul(
                        ob[:, q0:512],
                        v2e[p][:, k, 65:130],
                        pt2[:, 512 + q0 : 1024],
                        start=(k == 0),
                        stop=(k == nk - 1),
                    )
                    pt_tiles[k] = None

                for k in range(nk):
                    scores(k)
                    if k > 0:
                        av(k - 1)
                av(nk - 1)

                stage = pc.tile([65, 2, 512], F32, tag="stage", name="stage")
                nc.vector.tensor_copy(stage[:, 0, :], oa[:])
                nc.vector.tensor_copy(stage[:, 1, :], ob[:])
                nc.sync.dma_start(
                    out=out.rearrange("h (nl q) -> h nl q", nl=HPC)[
                        :, 2 * p : 2 * p + 2, qb * 512 : (qb + 1) * 512
                    ],
                    in_=stage[:],
                )

            # interleave: attention on early q-blocks overlaps the second
            # projection chunk (its exp work runs under proj matmuls)
            proj_chunk(0)
            attention(0, 1)
            attention(1, 1)
            attention(0, 0)
            attention(1, 0)
            proj_chunk(1)
            attention(0, 3)
            attention(1, 3)
            attention(0, 2)
            attention(1, 2)

    _split_excess_waits(nc)
    return nc


_NC_CACHE = None


def _get_nc():
    global _NC_CACHE
    if _NC_CACHE is None:
        _NC_CACHE = build_nc()
    return _NC_CACHE


def _host_inputs(x, W_qkv):
    """Per-core input maps."""
    xT = [np.ascontiguousarray(x[b].T).astype(NPD) for b in range(B)]  # [D, S]
    # w[pair, t, dtile, 128, 128]: cols 0:64 head a, 64:128 head b
    Wr = np.ascontiguousarray(W_qkv.reshape(NH, DT, 128, 3, HD))
    ki = np.arange(128)[:, None]
    qj = np.arange(512)[None, :]
    m1 = np.zeros((4, 128, 512), dtype=np.float32)
    for r in range(4):
        m1[r] = (ki <= qj - 128 * r).astype(np.float32)
    mask = np.concatenate([m1, m1], axis=2).astype(NPD)  # [4, 128, 1024]
    ident = np.eye(128, dtype=np.float32).astype(NPD)
    in_maps = []
    for c in range(NCORES):
        b = c // 4
        h0 = 4 * (c % 4)
        w = np.empty((NPAIR, 3, DT, 128, 128), dtype=np.float32)
        for p in range(NPAIR):
            ha, hb = h0 + 2 * p, h0 + 2 * p + 1
            for t in range(3):
                w[p, t, :, :, 0:64] = Wr[ha, :, :, t, :]
                w[p, t, :, :, 64:128] = Wr[hb, :, :, t, :]
        in_maps.append(
            {"xT": xT[b], "w": w.astype(NPD), "mask": mask, "ident": ident}
        )
    return in_maps


def _host_epilogue(results, W_out):
    W_sum = W_out.sum(axis=(0, 1)).astype(np.float32)  # [D]
    O = np.empty((B, NH, S, HD), dtype=np.float32)
    for c in range(NCORES):
        o = results[c]["out"]  # [65, 4*2048]
        b = c // 4
        h0 = 4 * (c % 4)
        body = o[0:64].reshape(64, HPC, S)  # [h, nl, s]
        den = o[64].reshape(HPC, S)  # [nl, s]
        O[b, h0 : h0 + HPC] = body.transpose(1, 2, 0) / den[:, :, None]
    out2 = O.reshape(B, D, S)  # raw row-major reshape, as in the reference
    return np.ascontiguousarray(
        out2.transpose(0, 2, 1) * W_sum[None, None, :]
    ).astype(np.float32)


def _run(x, W_qkv, W_out, trace=False):
    nc = _get_nc()
    in_maps = _host_inputs(x, W_qkv)
    res = run_bass_kernel_spmd(
        nc,
        in_maps,
        list(range(NCORES)),
        trace=trace,
        trace_cores=list(range(NCORES)) if trace else None,
    )
    return _host_epilogue(res.results, W_out), res


def kernel(x, W_qkv, W_out):
    x = np.asarray(x, dtype=np.float32)
    W_qkv = np.asarray(W_qkv, dtype=np.float32)
    W_out = np.asarray(W_out, dtype=np.float32)
    out, _ = _run(x, W_qkv, W_out, trace=False)
    return out


def kernel_traced(x, W_qkv, W_out):
    out, res = _run(
        np.asarray(x, np.float32),
        np.asarray(W_qkv, np.float32),
        np.asarray(W_out, np.float32),
        trace=True,
    )
    return out, res


# revision 16
# speedup vs baseline: 1.1428x; 1.0122x over previous
"""Causal multi-head attention kernel for TRN2 (8 NeuronCores, SPMD).

Problem: x[2,2048,1024], per-head W_qkv[16,1024,192], W_out[16,64,1024].
  qkv = einsum('bsd,ndh->bnsh', x, W_qkv); causal softmax attention per head;
  out.reshape(B,-1,S); einsum('bds,nhd->bsd', out, W_out).

Key observation: the final einsum does NOT contract d (it appears in both
operands and the output), so it reduces to
  result[b,s,d] = out_reshaped[b,d,s] * W_sum[d],  W_sum[d] = sum_{n,h} W_out[n,h,d]
i.e. a raw reshape + transpose + per-column scale. That part is pure data
movement and is done on the host; the device computes the attention.

Sharding: 2 batches x 16 heads = 32 jobs; core c handles batch c//4 and the
4 heads [4*(c%4), 4*(c%4)+4), as 2 head-pairs packed into 128 partitions.

Device per core (matmuls in fp16: full PE rate, ~16x better element
precision than bf16; PSUM accumulation is fp32):
  - QKV projection: psum = sum_d W2[d].T @ xT[d], 2 heads packed in M;
    two q-chunks share one weight load.
  - K^T kept head-packed [2H=128, S].  Q^T stored zero-padded per head
    (head a in rows 0:64 + zero rows, head b in rows 64:128 + zero rows)
    so each score matmul is a canonical full-K=128 matmul whose stationary
    operand (the packed K^T tile) is shared by both heads.
  - V^T -> [k, Va|1|Vb|1] tiles via PE transpose; the appended ones-column
    makes the AV matmul also produce the softmax denominator.
  - scores: S^T[k,q] tile pair for both heads in one 2-bank PSUM tile;
    ONE exp (ScalarE, scale=1/8, no max-subtraction needed: scores~N(0,1))
    per k-step; causal crossing tiles masked by a 0/1 fp16 multiply.
  - O'^T[65,q] += [V|1].T @ P^T accumulated over k: rows 0..63 attention
    output, row 64 denominator.  Causal column trimming on all of
    scores/exp/AV.
Host epilogue: normalize, reshape, scale by W_sum.
"""

import numpy as np

import concourse.bass as bass
import concourse.mybir as mybir
from concourse.tile import TileContext
from concourse.bass_utils import run_bass_kernel_spmd

F32 = mybir.dt.float32
MMD = mybir.dt.float16  # matmul operand dtype
NPD = np.float16

B, S, D, NH, HD = 2, 2048, 1024, 16, 64  # batch, seq, model, heads, head_dim
NCORES = 8
HPC = 4  # heads per core
NPAIR = 2  # head pairs per core
DT = D // 128  # 8 D-tiles
NQB = S // 512  # 4 q blocks
NKT = S // 128  # 16 k tiles
SCALE = 1.0 / np.sqrt(HD)


def _split_excess_waits(nc, limit=1):
    """This walrus build rejects >1 sync-wait per instruction; hoist extra
    waits onto preceding same-engine no-ops (identical blocking semantics)."""
    cnt = 0
    for fn in nc.m.functions:
        for blk in fn.blocks:
            out = []
            for inst in blk.instructions:
                si = inst.sync_info
                if si is not None and si.on_wait and len(si.on_wait) > limit:
                    waits = list(si.on_wait)
                    excess, keep = waits[:-limit], waits[-limit:]
                    for i in range(0, len(excess), limit):
                        nop = mybir.InstNoOp(
                            name=f"wsplit_{cnt}", ins=[], outs=[], engine=inst.engine
                        )
                        cnt += 1
                        nop.sync_info = mybir.SyncInfo(
                            on_wait=excess[i : i + limit], on_update=[]
                        )
                        out.append(nop)
                    inst.sync_info = mybir.SyncInfo(
                        on_wait=keep, on_update=list(si.on_update or [])
                    )
                out.append(inst)
            blk.instructions = out
    return cnt


def build_nc():
    nc = bass.Bass()
    xT = nc.declare_dram_parameter("xT", [D, S], MMD, isOutput=False)
    w = nc.declare_dram_parameter("w", [NPAIR, 3, DT, 128, 128], MMD, isOutput=False)
    mask = nc.declare_dram_parameter("mask", [4, 128, 1024], MMD, isOutput=False)
    ident = nc.declare_dram_parameter("ident", [128, 128], MMD, isOutput=False)
    out = nc.declare_dram_parameter("out", [65, HPC * S], F32, isOutput=True)

    with TileContext(nc) as tc:
        with (
            tc.tile_pool(name="persist", bufs=1) as pp,
            tc.tile_pool(name="psum", bufs=3, space="PSUM") as ps,
            tc.tile_pool(name="work", bufs=2) as pc,
        ):
            # ---- persistent SBUF tensors (Q^T and K^T head-packed [2H, S])
            qt2 = [
                pp.tile([128, S], MMD, tag=f"qt{p}", name=f"qtt{p}")
                for p in range(NPAIR)
            ]
            kt2 = [
                pp.tile([128, S], MMD, tag=f"kt{p}", name=f"ktt{p}")
                for p in range(NPAIR)
            ]
            v2e = [
                pp.tile([128, NKT, 130], MMD, tag=f"v2e{p}", name=f"v2e{p}")
                for p in range(NPAIR)
            ]
            mask_sb = pp.tile([128, 4, 1024], MMD, tag="mask", name="mask_sb")
            ident_sb = pp.tile([128, 128], MMD, tag="ident", name="ident_sb")
            xt_sb = pp.tile([128, DT, S], MMD, tag="xt", name="xt_sb")
            w_sb = pp.tile([128, NPAIR * 3 * DT, 128], MMD, tag="w", name="w_sb")
            vt = [
                pp.tile([128, S], MMD, tag=f"vt{p}", name=f"vt{p}")
                for p in range(NPAIR)
            ]

            # DMA order = consumption order: pair-0 weights + first xT half
            # gate the first projection matmuls.
            w_v = w.rearrange("a t d k m -> k (a t d) m")
            xt_v = xT.rearrange("(dt p) s -> p dt s", p=128)
            nc.sync.dma_start(out=w_sb[:, 0 : 3 * DT, :], in_=w_v[:, 0 : 3 * DT, :])
            for c4 in range(2):
                cs = slice(c4 * 512, (c4 + 1) * 512)
                nc.sync.dma_start(out=xt_sb[:, :, cs], in_=xt_v[:, :, cs])
            nc.sync.dma_start(out=ident_sb[:], in_=ident[:])
            nc.sync.dma_start(
                out=w_sb[:, 3 * DT : 6 * DT, :], in_=w_v[:, 3 * DT : 6 * DT, :]
            )
            for c4 in range(2, 4):
                cs = slice(c4 * 512, (c4 + 1) * 512)
                nc.sync.dma_start(out=xt_sb[:, :, cs], in_=xt_v[:, :, cs])
            nc.sync.dma_start(out=mask_sb[:], in_=mask.rearrange("r k q -> k r q"))
            for p in range(NPAIR):
                nc.vector.memset(v2e[p][:, :, 64], 1.0)
                nc.vector.memset(v2e[p][:, :, 129], 1.0)

            def proj_chunk(qcp):
                """Project q-columns [qcp*1024, (qcp+1)*1024) for all pairs;
                transpose the V k-tiles of that chunk."""
                for p in range(NPAIR):
                    for t in range(3):
                        acc2 = ps.tile([128, 1024], F32, tag="mm", name="acc2")
                        for d in range(DT):
                            wsl = w_sb[:, (p * 3 + t) * DT + d, :]
                            for h in range(2):
                                qc = 2 * qcp + h
                                nc.tensor.matmul(
                                    acc2[:, h * 512 : (h + 1) * 512],
                                    wsl,
                                    xt_sb[:, d, qc * 512 : (qc + 1) * 512],
                                    start=(d == 0),
                                    stop=(d == DT - 1),
                                )
                        csl = slice(qcp * 1024, (qcp + 1) * 1024)
                        if t == 0:
                            nc.vector.tensor_copy(qt2[p][:, csl], acc2[:])
                        elif t == 1:
                            nc.vector.tensor_copy(kt2[p][:, csl], acc2[:])
                        else:  # V -> transpose this chunk's k-tiles now
                            nc.vector.tensor_copy(vt[p][:, csl], acc2[:])
                            for k in range(8 * qcp, 8 * qcp + 8):
                                tp = ps.tile([128, 512], MMD, tag="acca", name="tp", bufs=1)
                                nc.tensor.transpose(
                                    tp[:, 0:128],
                                    vt[p][:, k * 128 : (k + 1) * 128],
                                    ident_sb[:],
                                )
                                nc.vector.tensor_copy(
                                    v2e[p][:, k, 0:64], tp[:, 0:64]
                                )
                                nc.vector.tensor_copy(
                                    v2e[p][:, k, 65:129], tp[:, 64:128]
                                )

            def attention(p, qb):
                nk = 4 * (qb + 1)  # causal: k tiles 0..nk-1
                oa = ps.tile([65, 512], F32, tag="acca", name="oa", bufs=1)
                ob = ps.tile([65, 512], F32, tag="accb", name="ob", bufs=1)
                # software pipeline: scores(k) ... AV(k-1) so the PE never
                # stalls on the ScalarE exp of the current tile
                pt_tiles = [None] * nk

                def scores(k):
                    q0 = max(0, 128 * (k - 4 * qb))
                    s2 = ps.tile([128, 1024], F32, tag="mm", name="s2")
                    qsl = slice(qb * 512 + q0, (qb + 1) * 512)
                    for e in range(2):
                        rows = slice(64 * e, 64 * e + 64)
                        nc.tensor.matmul(
                            s2[:, e * 512 + q0 : (e + 1) * 512],
                            kt2[p][rows, k * 128 : (k + 1) * 128],
                            qt2[p][rows, qsl],
                            start=True,
                            stop=True,
                            tile_position=(64 * e, 0),
                        )
                    pt2 = pc.tile([128, 1024], MMD, tag="pt", name="pt2", bufs=4)
                    nc.scalar.activation(
                        pt2[:, q0:1024],
                        s2[:, q0:1024],
                        mybir.ActivationFunctionType.Exp,
                        scale=float(SCALE),
                    )
                    rel = k - 4 * qb
                    if rel >= 0:  # diagonal-crossing: 0/1 mask
                        nc.vector.tensor_mul(
                            pt2[:, q0:1024],
                            pt2[:, q0:1024],
                            mask_sb[:, rel, q0:1024],
                        )
                    pt_tiles[k] = (pt2, q0)

                def av(k):
                    pt2, q0 = pt_tiles[k]
                    nc.tensor.matmul(
                        oa[:, q0:512],
                        v2e[p][:, k, 0:65],
                        pt2[:, q0:512],
                        start=(k == 0),
                        stop=(k == nk - 1),
                    )
                    nc.tensor.matmul(
                        ob[:, q0:512],
                        v2e[p][:, k, 65:130],
                        pt2[:, 512 + q0 : 1024],
                        start=(k == 0),
                        stop=(k == nk - 1),
                    )
                    pt_tiles[k] = None

                for k in range(nk):
                    scores(k)
                    if k > 0:
                        av(k - 1)
                av(nk - 1)

                stage = pc.tile([65, 2, 512], F32, tag="stage", name="stage")
                nc.vector.tensor_copy(stage[:, 0, :], oa[:])
                nc.vector.tensor_copy(stage[:, 1, :], ob[:])
                nc.sync.dma_start(
                    out=out.rearrange("h (nl q) -> h nl q", nl=HPC)[
                        :, 2 * p : 2 * p + 2, qb * 512 : (qb + 1) * 512
                    ],
                    in_=stage[:],
                )

            # interleave: attention on early q-blocks overlaps the second
            # projection chunk (its exp work runs under proj matmuls)
            proj_chunk(0)
            attention(0, 1)
            attention(1, 1)
            attention(0, 0)
            attention(1, 0)
            proj_chunk(1)
            attention(0, 3)
            attention(1, 3)
            attention(0, 2)
            attention(1, 2)

    _split_excess_waits(nc)
    return nc


_NC_CACHE = None


def _get_nc():
    global _NC_CACHE
    if _NC_CACHE is None:
        _NC_CACHE = build_nc()
    return _NC_CACHE


def _host_inputs(x, W_qkv):
    """Per-core input maps."""
    xT = [np.ascontiguousarray(x[b].T).astype(NPD) for b in range(B)]  # [D, S]
    # w[pair, t, dtile, 128, 128]: cols 0:64 head a, 64:128 head b
    Wr = np.ascontiguousarray(W_qkv.reshape(NH, DT, 128, 3, HD))
    ki = np.arange(128)[:, None]
    qj = np.arange(512)[None, :]
    m1 = np.zeros((4, 128, 512), dtype=np.float32)
    for r in range(4):
        m1[r] = (ki <= qj - 128 * r).astype(np.float32)
    mask = np.concatenate([m1, m1], axis=2).astype(NPD)  # [4, 128, 1024]
    ident = np.eye(128, dtype=np.float32).astype(NPD)
    in_maps = []
    for c in range(NCORES):
        b = c // 4
        h0 = 4 * (c % 4)
        w = np.empty((NPAIR, 3, DT, 128, 128), dtype=np.float32)
        for p in range(NPAIR):
            ha, hb = h0 + 2 * p, h0 + 2 * p + 1
            for t in range(3):
                w[p, t, :, :, 0:64] = Wr[ha, :, :, t, :]
                w[p, t, :, :, 64:128] = Wr[hb, :, :, t, :]
        in_maps.append(
            {"xT": xT[b], "w": w.astype(NPD), "mask": mask, "ident": ident}
        )
    return in_maps


def _host_epilogue(results, W_out):
    W_sum = W_out.sum(axis=(0, 1)).astype(np.float32)  # [D]
    O = np.empty((B, NH, S, HD), dtype=np.float32)
    for c in range(NCORES):
        o = results[c]["out"]  # [65, 4*2048]
        b = c // 4
        h0 = 4 * (c % 4)
        body = o[0:64].reshape(64, HPC, S)  # [h, nl, s]
        den = o[64].reshape(HPC, S)  # [nl, s]
        O[b, h0 : h0 + HPC] = body.transpose(1, 2, 0) / den[:, :, None]
    out2 = O.reshape(B, D, S)  # raw row-major reshape, as in the reference
    return np.ascontiguousarray(
        out2.transpose(0, 2, 1) * W_sum[None, None, :]
    ).astype(np.float32)


def _run(x, W_qkv, W_out, trace=False):
    nc = _get_nc()
    in_maps = _host_inputs(x, W_qkv)
    res = run_bass_kernel_spmd(
        nc,
        in_maps,
        list(range(NCORES)),
        trace=trace,
        trace_cores=list(range(NCORES)) if trace else None,
    )
    return _host_epilogue(res.results, W_out), res


def kernel(x, W_qkv, W_out):
    x = np.asarray(x, dtype=np.float32)
    W_qkv = np.asarray(W_qkv, dtype=np.float32)
    W_out = np.asarray(W_out, dtype=np.float32)
    out, _ = _run(x, W_qkv, W_out, trace=False)
    return out


def kernel_traced(x, W_qkv, W_out):
    out, res = _run(
        np.asarray(x, np.float32),
        np.asarray(W_qkv, np.float32),
        np.asarray(W_out, np.float32),
        trace=True,
    )
    return out, res


# revision 17
# speedup vs baseline: 1.1460x; 1.0028x over previous
"""Causal multi-head attention kernel for TRN2 (8 NeuronCores, SPMD).

Problem: x[2,2048,1024], per-head W_qkv[16,1024,192], W_out[16,64,1024].
  qkv = einsum('bsd,ndh->bnsh', x, W_qkv); causal softmax attention per head;
  out.reshape(B,-1,S); einsum('bds,nhd->bsd', out, W_out).

Key observation: the final einsum does NOT contract d (it appears in both
operands and the output), so it reduces to
  result[b,s,d] = out_reshaped[b,d,s] * W_sum[d],  W_sum[d] = sum_{n,h} W_out[n,h,d]
i.e. a raw reshape + transpose + per-column scale. That part is pure data
movement and is done on the host; the device computes the attention.

Sharding: 2 batches x 16 heads = 32 jobs; core c handles batch c//4 and the
4 heads [4*(c%4), 4*(c%4)+4), as 2 head-pairs packed into 128 partitions.

Device per core (matmuls in fp16: full PE rate, ~16x better element
precision than bf16; PSUM accumulation is fp32):
  - QKV projection: psum = sum_d W2[d].T @ xT[d], 2 heads packed in M;
    two q-chunks share one weight load.
  - K^T kept head-packed [2H=128, S].  Q^T stored zero-padded per head
    (head a in rows 0:64 + zero rows, head b in rows 64:128 + zero rows)
    so each score matmul is a canonical full-K=128 matmul whose stationary
    operand (the packed K^T tile) is shared by both heads.
  - V^T -> [k, Va|1|Vb|1] tiles via PE transpose; the appended ones-column
    makes the AV matmul also produce the softmax denominator.
  - scores: S^T[k,q] tile pair for both heads in one 2-bank PSUM tile;
    ONE exp (ScalarE, scale=1/8, no max-subtraction needed: scores~N(0,1))
    per k-step; causal crossing tiles masked by a 0/1 fp16 multiply.
  - O'^T[65,q] += [V|1].T @ P^T accumulated over k: rows 0..63 attention
    output, row 64 denominator.  Causal column trimming on all of
    scores/exp/AV.
Host epilogue: normalize, reshape, scale by W_sum.
"""

import numpy as np

import concourse.bass as bass
import concourse.mybir as mybir
from concourse.tile import TileContext
from concourse.bass_utils import run_bass_kernel_spmd

F32 = mybir.dt.float32
MMD = mybir.dt.float16  # matmul operand dtype
NPD = np.float16

B, S, D, NH, HD = 2, 2048, 1024, 16, 64  # batch, seq, model, heads, head_dim
NCORES = 8
HPC = 4  # heads per core
NPAIR = 2  # head pairs per core
DT = D // 128  # 8 D-tiles
NQB = S // 512  # 4 q blocks
NKT = S // 128  # 16 k tiles
SCALE = 1.0 / np.sqrt(HD)


def _split_excess_waits(nc, limit=1):
    """This walrus build rejects >1 sync-wait per instruction; hoist extra
    waits onto preceding same-engine no-ops (identical blocking semantics)."""
    cnt = 0
    for fn in nc.m.functions:
        for blk in fn.blocks:
            out = []
            for inst in blk.instructions:
                si = inst.sync_info
                if si is not None and si.on_wait and len(si.on_wait) > limit:
                    waits = list(si.on_wait)
                    excess, keep = waits[:-limit], waits[-limit:]
                    for i in range(0, len(excess), limit):
                        nop = mybir.InstNoOp(
                            name=f"wsplit_{cnt}", ins=[], outs=[], engine=inst.engine
                        )
                        cnt += 1
                        nop.sync_info = mybir.SyncInfo(
                            on_wait=excess[i : i + limit], on_update=[]
                        )
                        out.append(nop)
                    inst.sync_info = mybir.SyncInfo(
                        on_wait=keep, on_update=list(si.on_update or [])
                    )
                out.append(inst)
            blk.instructions = out
    return cnt


def build_nc():
    nc = bass.Bass()
    xT = nc.declare_dram_parameter("xT", [D, S], MMD, isOutput=False)
    w = nc.declare_dram_parameter("w", [NPAIR, 3, DT, 128, 128], MMD, isOutput=False)
    mask = nc.declare_dram_parameter("mask", [4, 128, 1024], MMD, isOutput=False)
    ident = nc.declare_dram_parameter("ident", [128, 128], MMD, isOutput=False)
    out = nc.declare_dram_parameter("out", [65, HPC * S], F32, isOutput=True)

    with TileContext(nc) as tc:
        with (
            tc.tile_pool(name="persist", bufs=1) as pp,
            tc.tile_pool(name="psum", bufs=3, space="PSUM") as ps,
            tc.tile_pool(name="work", bufs=2) as pc,
        ):
            # ---- persistent SBUF tensors (Q^T and K^T head-packed [2H, S])
            qt2 = [
                pp.tile([128, S], MMD, tag=f"qt{p}", name=f"qtt{p}")
                for p in range(NPAIR)
            ]
            kt2 = [
                pp.tile([128, S], MMD, tag=f"kt{p}", name=f"ktt{p}")
                for p in range(NPAIR)
            ]
            v2e = [
                pp.tile([128, NKT, 130], MMD, tag=f"v2e{p}", name=f"v2e{p}")
                for p in range(NPAIR)
            ]
            mask_sb = pp.tile([128, 4, 1024], MMD, tag="mask", name="mask_sb")
            ident_sb = pp.tile([128, 128], MMD, tag="ident", name="ident_sb")
            xt_sb = pp.tile([128, DT, S], MMD, tag="xt", name="xt_sb")
            w_sb = pp.tile([128, NPAIR * 3 * DT, 128], MMD, tag="w", name="w_sb")
            vt = [
                pp.tile([128, S], MMD, tag=f"vt{p}", name=f"vt{p}")
                for p in range(NPAIR)
            ]

            # DMA order = consumption order: pair-0 weights + first xT half
            # gate the first projection matmuls.
            w_v = w.rearrange("a t d k m -> k (a t d) m")
            xt_v = xT.rearrange("(dt p) s -> p dt s", p=128)
            nc.sync.dma_start(out=w_sb[:, 0 : 3 * DT, :], in_=w_v[:, 0 : 3 * DT, :])
            for c4 in range(2):
                cs = slice(c4 * 512, (c4 + 1) * 512)
                nc.sync.dma_start(out=xt_sb[:, :, cs], in_=xt_v[:, :, cs])
            nc.sync.dma_start(out=ident_sb[:], in_=ident[:])
            nc.sync.dma_start(
                out=w_sb[:, 3 * DT : 6 * DT, :], in_=w_v[:, 3 * DT : 6 * DT, :]
            )
            for c4 in range(2, 4):
                cs = slice(c4 * 512, (c4 + 1) * 512)
                nc.sync.dma_start(out=xt_sb[:, :, cs], in_=xt_v[:, :, cs])
            nc.sync.dma_start(out=mask_sb[:], in_=mask.rearrange("r k q -> k r q"))
            for p in range(NPAIR):
                nc.vector.memset(v2e[p][:, :, 64], 1.0)
                nc.vector.memset(v2e[p][:, :, 129], 1.0)

            def proj_step(qcp, p, t):
                """Project one (pair, qkv-type) for q-columns
                [qcp*1024, (qcp+1)*1024); transpose V k-tiles of the chunk."""
                if True:
                    if True:
                        acc2 = ps.tile([128, 1024], F32, tag="mm", name="acc2")
                        for d in range(DT):
                            wsl = w_sb[:, (p * 3 + t) * DT + d, :]
                            for h in range(2):
                                qc = 2 * qcp + h
                                nc.tensor.matmul(
                                    acc2[:, h * 512 : (h + 1) * 512],
                                    wsl,
                                    xt_sb[:, d, qc * 512 : (qc + 1) * 512],
                                    start=(d == 0),
                                    stop=(d == DT - 1),
                                )
                        csl = slice(qcp * 1024, (qcp + 1) * 1024)
                        if t == 0:
                            nc.vector.tensor_copy(qt2[p][:, csl], acc2[:])
                        elif t == 1:
                            nc.vector.tensor_copy(kt2[p][:, csl], acc2[:])
                        else:  # V -> transpose this chunk's k-tiles now
                            nc.vector.tensor_copy(vt[p][:, csl], acc2[:])
                            for k in range(8 * qcp, 8 * qcp + 8):
                                tp = ps.tile([128, 512], MMD, tag="acca", name="tp", bufs=1)
                                nc.tensor.transpose(
                                    tp[:, 0:128],
                                    vt[p][:, k * 128 : (k + 1) * 128],
                                    ident_sb[:],
                                )
                                nc.vector.tensor_copy(
                                    v2e[p][:, k, 0:64], tp[:, 0:64]
                                )
                                nc.vector.tensor_copy(
                                    v2e[p][:, k, 65:129], tp[:, 64:128]
                                )

            def attention(p, qb):
                nk = 4 * (qb + 1)  # causal: k tiles 0..nk-1
                oa = ps.tile([65, 512], F32, tag="acca", name="oa", bufs=1)
                ob = ps.tile([65, 512], F32, tag="accb", name="ob", bufs=1)
                # software pipeline: scores(k) ... AV(k-1) so the PE never
                # stalls on the ScalarE exp of the current tile
                pt_tiles = [None] * nk

                def scores(k):
                    q0 = max(0, 128 * (k - 4 * qb))
                    s2 = ps.tile([128, 1024], F32, tag="mm", name="s2")
                    qsl = slice(qb * 512 + q0, (qb + 1) * 512)
                    for e in range(2):
                        rows = slice(64 * e, 64 * e + 64)
                        nc.tensor.matmul(
                            s2[:, e * 512 + q0 : (e + 1) * 512],
                            kt2[p][rows, k * 128 : (k + 1) * 128],
                            qt2[p][rows, qsl],
                            start=True,
                            stop=True,
                            tile_position=(64 * e, 0),
                        )
                    pt2 = pc.tile([128, 1024], MMD, tag="pt", name="pt2", bufs=4)
                    nc.scalar.activation(
                        pt2[:, q0:1024],
                        s2[:, q0:1024],
                        mybir.ActivationFunctionType.Exp,
                        scale=float(SCALE),
                    )
                    rel = k - 4 * qb
                    if rel >= 0:  # diagonal-crossing: 0/1 mask
                        nc.vector.tensor_mul(
                            pt2[:, q0:1024],
                            pt2[:, q0:1024],
                            mask_sb[:, rel, q0:1024],
                        )
                    pt_tiles[k] = (pt2, q0)

                def av(k):
                    pt2, q0 = pt_tiles[k]
                    nc.tensor.matmul(
                        oa[:, q0:512],
                        v2e[p][:, k, 0:65],
                        pt2[:, q0:512],
                        start=(k == 0),
                        stop=(k == nk - 1),
                    )
                    nc.tensor.matmul(
                        ob[:, q0:512],
                        v2e[p][:, k, 65:130],
                        pt2[:, 512 + q0 : 1024],
                        start=(k == 0),
                        stop=(k == nk - 1),
                    )
                    pt_tiles[k] = None

                for k in range(nk):
                    scores(k)
                    if k > 0:
                        av(k - 1)
                av(nk - 1)

                stage = pc.tile([65, 2, 512], F32, tag="stage", name="stage")
                nc.vector.tensor_copy(stage[:, 0, :], oa[:])
                nc.vector.tensor_copy(stage[:, 1, :], ob[:])
                nc.sync.dma_start(
                    out=out.rearrange("h (nl q) -> h nl q", nl=HPC)[
                        :, 2 * p : 2 * p + 2, qb * 512 : (qb + 1) * 512
                    ],
                    in_=stage[:],
                )

            # interleave: the second projection chunk's steps are spread
            # between the early attention blocks so projection matmuls fill
            # the PE while ScalarE drains the attention exps (and vice versa)
            for p in range(NPAIR):
                for t in range(3):
                    proj_step(0, p, t)
            attention(0, 1)
            proj_step(1, 0, 0)
            attention(1, 1)
            proj_step(1, 0, 1)
            attention(0, 0)
            proj_step(1, 0, 2)
            attention(1, 0)
            proj_step(1, 1, 0)
            proj_step(1, 1, 1)
            proj_step(1, 1, 2)
            attention(0, 3)
            attention(1, 3)
            attention(0, 2)
            attention(1, 2)

    _split_excess_waits(nc)
    return nc


_NC_CACHE = None


def _get_nc():
    global _NC_CACHE
    if _NC_CACHE is None:
        _NC_CACHE = build_nc()
    return _NC_CACHE


def _host_inputs(x, W_qkv):
    """Per-core input maps."""
    xT = [np.ascontiguousarray(x[b].T).astype(NPD) for b in range(B)]  # [D, S]
    # w[pair, t, dtile, 128, 128]: cols 0:64 head a, 64:128 head b
    Wr = np.ascontiguousarray(W_qkv.reshape(NH, DT, 128, 3, HD))
    ki = np.arange(128)[:, None]
    qj = np.arange(512)[None, :]
    m1 = np.zeros((4, 128, 512), dtype=np.float32)
    for r in range(4):
        m1[r] = (ki <= qj - 128 * r).astype(np.float32)
    mask = np.concatenate([m1, m1], axis=2).astype(NPD)  # [4, 128, 1024]
    ident = np.eye(128, dtype=np.float32).astype(NPD)
    in_maps = []
    for c in range(NCORES):
        b = c // 4
        h0 = 4 * (c % 4)
        w = np.empty((NPAIR, 3, DT, 128, 128), dtype=np.float32)
        for p in range(NPAIR):
            ha, hb = h0 + 2 * p, h0 + 2 * p + 1
            for t in range(3):
                w[p, t, :, :, 0:64] = Wr[ha, :, :, t, :]
                w[p, t, :, :, 64:128] = Wr[hb, :, :, t, :]
        in_maps.append(
            {"xT": xT[b], "w": w.astype(NPD), "mask": mask, "ident": ident}
        )
    return in_maps


def _host_epilogue(results, W_out):
    W_sum = W_out.sum(axis=(0, 1)).astype(np.float32)  # [D]
    O = np.empty((B, NH, S, HD), dtype=np.float32)
    for c in range(NCORES):
        o = results[c]["out"]  # [65, 4*2048]
        b = c // 4
        h0 = 4 * (c % 4)
        body = o[0:64].reshape(64, HPC, S)  # [h, nl, s]
        den = o[64].reshape(HPC, S)  # [nl, s]
        O[b, h0 : h0 + HPC] = body.transpose(1, 2, 0) / den[:, :, None]
    out2 = O.reshape(B, D, S)  # raw row-major reshape, as in the reference
    return np.ascontiguousarray(
        out2.transpose(0, 2, 1) * W_sum[None, None, :]
    ).astype(np.float32)


def _run(x, W_qkv, W_out, trace=False):
    nc = _get_nc()
    in_maps = _host_inputs(x, W_qkv)
    res = run_bass_kernel_spmd(
        nc,
        in_maps,
        list(range(NCORES)),
        trace=trace,
        trace_cores=list(range(NCORES)) if trace else None,
    )
    return _host_epilogue(res.results, W_out), res


def kernel(x, W_qkv, W_out):
    x = np.asarray(x, dtype=np.float32)
    W_qkv = np.asarray(W_qkv, dtype=np.float32)
    W_out = np.asarray(W_out, dtype=np.float32)
    out, _ = _run(x, W_qkv, W_out, trace=False)
    return out


def kernel_traced(x, W_qkv, W_out):
    out, res = _run(
        np.asarray(x, np.float32),
        np.asarray(W_qkv, np.float32),
        np.asarray(W_out, np.float32),
        trace=True,
    )
    return out, res
